# revision 103
# baseline (speedup 1.0000x reference)
"""Multi-head attention (B=4, S=2048, C=1024, H=16) on 8 TRN2 NeuronCores.

Tensor-parallel over heads: core c owns head pair (2c, 2c+1) for ALL 4
batches and computes a partial out-projection y_c = O_c @ W_out[:, c*128:
(c+1)*128].T; the host sums the 8 partials and adds b_out + W_out @ b_v
(the V bias commutes through softmax+outproj since rows of P sum to 1).

Per core, everything is bf16 (PSUM accumulation in fp32) and SBUF-resident
per batch; x streams in per batch, double-buffered, 16 descriptors per
batch so one batch saturates all DMA queues before the next queues behind.
The per-batch pipeline keeps ScalarE (exp) saturated while the PE
interleaves, as cost-budgeted filler between score/PV blocks, the NEXT
batch's QKV projection and earlier batches' out-projections (deferred one
batch, with halves of outproj(0..1) held into batch 3's window, which has
no proj filler of its own):

  proj(b): qT/kT feature-major [128 pair-feats, 2048 rows] via W-stationary
  matmuls; V keys-major via x-stationary matmuls into a padded stationary
  layout [V_h(64) | 1 | 0*63] per (key-tile, head) so the PV matmul also
  accumulates the softmax denominator at out row 64.

  attention(b, ich of 512 q): per key-tile j: row-packed pair scores
  (head A on PE rows 0-63, head B on 64-127; the hardware runs them as
  concurrent 64-row groups, ~385ns/pair) into a double-buffered 2-bank
  PSUM tile -> one exp (scale=0.125 folded in) -> PV of tile j-1 pipelined
  behind the exp.  Per-ich normalize straight from the PV PSUM banks:
  VectorE copy of the denominator row, reciprocal_approx_fast ([1,512],
  ~18 bits), gpsimd partition_broadcast, one tensor mul into OT.  On the
  very last ich the denominator copies ride the then-idle ScalarE and the
  outproj casts alternate ScalarE/VectorE to shorten the exposed tail.

PSUM (8 banks): scores 2x2, PV accumulators 2x1, proj/outproj acc 2x1.
Mid-stream engine discipline (measured, not guessed): ScalarE's FIFO is
reserved for the exp stream (inserting copies/bias-adds costs ~14us in
score-bank stalls); deferred outproj runs as contiguous blocks because
proj closures hold PSUM accs across 4-8 closures (+60us if interleaved).
"""

from collections import deque
from contextlib import ExitStack

import numpy as np
import ml_dtypes

import concourse.mybir as mybir
import concourse.tile as tile
from concourse import bacc
from concourse.bass_utils import run_bass_kernel_spmd
from concourse.masks import make_identity

F32 = mybir.dt.float32
BF16 = mybir.dt.bfloat16
AF = mybir.ActivationFunctionType

B, S, C, H, DH = 4, 2048, 1024, 16, 64
NCORES = 8
SCALE = DH ** -0.5  # 0.125
CT = C // 128  # 8 channel tiles
ST = S // 128  # 16 key tiles
NICH = S // 512  # 4 query chunks per batch


def build():
    nc = bacc.Bacc("TRN2", target_bir_lowering=False, debug=False,
                   num_devices=NCORES)

    # host-prepared layouts (pure data movement on the host):
    #   xTd[b, ct, p, s] = x[b, s, ct*128+p]
    #   wq[p, ct*128+f] = W_qkv[c*128+f, ct*128+p]          (this core's Q)
    #   wk / wv same with row offsets 1024+c*128 / 2048+c*128
    #   wo[p, ch] = W_out[ch, c*128+p]
    #   bq/bk/bv[p, 0] = b_qkv[(0|1024|2048) + c*128 + p]
    xTd = nc.dram_tensor("xT", [B, CT, 128, S], BF16, kind="ExternalInput").ap()
    wqd = nc.dram_tensor("wq", [128, CT * 128], BF16, kind="ExternalInput").ap()
    wkd = nc.dram_tensor("wk", [128, CT * 128], BF16, kind="ExternalInput").ap()
    wvd = nc.dram_tensor("wv", [128, CT * 128], BF16, kind="ExternalInput").ap()
    wod = nc.dram_tensor("wo", [128, C], BF16, kind="ExternalInput").ap()
    bqd = nc.dram_tensor("bq", [128, 1], F32, kind="ExternalInput").ap()
    bkd = nc.dram_tensor("bk", [128, 1], F32, kind="ExternalInput").ap()
    ypd = nc.dram_tensor("yp", [B * S, C], BF16, kind="ExternalOutput").ap()

    with tile.TileContext(nc) as tc, ExitStack() as ctx:
        const = ctx.enter_context(tc.tile_pool(name="const", bufs=1))
        ident = const.tile([128, 128], F32)
        make_identity(nc, ident[:])
        wsq = const.tile([128, CT * 128], BF16)
        wsk = const.tile([128, CT * 128], BF16)
        wsv = const.tile([128, CT * 128], BF16)
        wo = const.tile([128, C], BF16)
        bq = const.tile([128, 1], F32)
        bk = const.tile([128, 1], F32)
        for dst, src in ((wsk, wkd), (wsq, wqd), (bq, bqd), (bk, bkd)):
            nc.sync.dma_start(dst[:], src)

        # warmup tile for PE clock ramp during the startup DMA window
        wrm = const.tile([128, 512], BF16)
        for i in range(4):
            nc.vector.tensor_scalar(
                wrm[:, i * 128:(i + 1) * 128], ident[:, 0:128],
                0.0, 0.0, mybir.AluOpType.mult, mybir.AluOpType.mult)

        # persistent V tiles (ping-pong): ones/zeros pad written once
        vt_pair = [const.tile([128, ST * 256], BF16, name=f"vt{i}")
                   for i in range(2)]
        vt4_pair = []
        for vt in vt_pair:
            vt4 = vt[:].rearrange("p (t g f) -> p t g f", g=2, f=128)
            vt4_pair.append(vt4)
            nc.vector.tensor_scalar(
                vt4[:, :, :, DH:DH + 1],
                ident[:, 0:2 * ST].rearrange("p (t g) -> p t g", g=2),
                0.0, 1.0, mybir.AluOpType.mult, mybir.AluOpType.add)
            for g in range(2):
                nc.vector.tensor_scalar(
                    vt4[:, :, g:g + 1, DH + 1:128],
                    wsk[:, 0:ST * (127 - DH)].rearrange(
                        "p (t g f) -> p t g f", g=1, f=127 - DH),
                    0.0, 0.0, mybir.AluOpType.mult, mybir.AluOpType.mult)

        xp = ctx.enter_context(tc.tile_pool(name="xp", bufs=2))
        qp = ctx.enter_context(tc.tile_pool(name="qp", bufs=2))
        kp = ctx.enter_context(tc.tile_pool(name="kp", bufs=2))
        # bufs=4: all four OT tiles stay alive (outproj(0),(1) halves are
        # held back into batch 3's filler window)
        otp = ctx.enter_context(tc.tile_pool(name="otp", bufs=4))
        pgp = ctx.enter_context(tc.tile_pool(name="pgp", bufs=5))
        rcpp = ctx.enter_context(tc.tile_pool(name="rcpp", bufs=8))
        rbp = ctx.enter_context(tc.tile_pool(name="rbp", bufs=6))
        # deep yb pool: y DMAs share the 16 round-robin queues with the 4MB
        # x prefetches, so several y writes can be in flight behind a
        # prefetch segment; a shallow pool stalls the cast->acc rotation
        ybp = ctx.enter_context(tc.tile_pool(name="ybp", bufs=12))
        sc_ps = ctx.enter_context(
            tc.tile_pool(name="sc_ps", bufs=2, space="PSUM"))
        pv_ps = ctx.enter_context(
            tc.tile_pool(name="pv_ps", bufs=1, space="PSUM"))
        acc_ps = ctx.enter_context(
            tc.tile_pool(name="acc_ps", bufs=2, space="PSUM"))

        xts = [None] * (B + 1)
        qTs = [None] * B
        kTs = [None] * B
        OTs = [None] * B

        def emit_x_dma(b, engines=None):
            # 16 descriptors spread one batch's x across all 16 round-robin
            # DMA queues.  Descriptor ENQUEUE itself costs ~0.6us on the
            # issuing engine, so the startup batches alternate between the
            # two hardware-DGE engines (Sync + the then-idle ScalarE) and
            # mid-stream prefetches go through gpsimd's software DGE to keep
            # Sync free for the latency-sensitive y-output descriptors.
            engines = engines or [nc.sync]
            xt = xp.tile([128, CT * S], BF16)
            xts[b] = xt
            n = 0
            for ct in range(CT):
                for hf in range(2):
                    lo = ct * S + hf * (S // 2)
                    engines[n % len(engines)].dma_start(
                        xt[:, lo:lo + S // 2],
                        xTd[b, ct, :, hf * (S // 2):(hf + 1) * (S // 2)])
                    n += 1

        # ---- projection of batch b, as lists of PE-sized closures ----
        def make_proj_closures(b):
            xt = xts[b]
            qT = qp.tile([128, S], BF16)
            kT = kp.tile([128, S], BF16)
            qTs[b], kTs[b] = qT, kT
            vt4 = vt4_pair[b % 2]

            # Q/K: per (which, rch) one 8-ct accumulation, split into 4
            # ~0.45us closures so the filler pacing stays fine-grained
            def qk_quads(wt, dst, bias, rch):
                st8 = {}
                def qk_u(q4):
                    def f():
                        if q4 == 0:
                            st8["acc"] = acc_ps.tile([128, 512], F32,
                                                     name="qkacc", tag="acc")
                        acc = st8["acc"]
                        for ct in range(2 * q4, 2 * q4 + 2):
                            nc.tensor.matmul(
                                acc[:], wt[:, ct * 128:(ct + 1) * 128],
                                xt[:, ct * S + rch * 512: ct * S + rch * 512 + 512],
                                start=(ct == 0), stop=(ct == 7))
                        if q4 == 3:
                            nc.vector.tensor_scalar_add(
                                dst[:, rch * 512:(rch + 1) * 512], acc[:],
                                bias[:, 0:1])
                    return f
                return [qk_u(q4) for q4 in range(4)]

            k_cls = [(C_QK, c)
                     for rch in range(4) for c in qk_quads(wsk, kT, bk, rch)]
            q_cls = [[(C_QK, c) for c in qk_quads(wsq, qT, bq, rch)]
                     for rch in range(4)]

            # V: per group of 4 key-tiles, 8 closures of 4-ct halves
            def v_group(g4):
                st8 = {}
                def v_u(u, h):
                    def f():
                        if u == 0 and h == 0:
                            st8["acc"] = acc_ps.tile([128, 512], F32,
                                                     name="vacc", tag="acc")
                        acc = st8["acc"]
                        kt = g4 * 4 + u
                        for ct in range(4 * h, 4 * h + 4):
                            nc.tensor.matmul(
                                acc[:, u * 128:(u + 1) * 128],
                                xt[:, ct * S + kt * 128: ct * S + (kt + 1) * 128],
                                wsv[:, ct * 128:(ct + 1) * 128],
                                start=(ct == 0), stop=(ct == CT - 1))
                        if u == 3 and h == 1:
                            nc.vector.tensor_copy(
                                vt4[:, g4 * 4:(g4 + 1) * 4, :, 0:DH],
                                acc[:].rearrange("p (u g f) -> p u g f",
                                                 u=4, f=DH))
                    return f
                return [v_u(u, h) for u in range(4) for h in range(2)]

            v_cls = [[(C_V, c) for c in v_group(g4)] for g4 in range(4)]
            return k_cls, q_cls, v_cls

        # ---- out-projection of one ich of batch b as closures ----
        # final=True (very last ich): alternate the PSUM->SBUF casts between
        # ScalarE (idle after the last exp; Copy shares the exp table) and
        # VectorE so the 2-deep acc rotation isn't serialized on one engine.
        def make_outproj_closures(b, ich, final=False):
            OT = OTs[b]
            # Cast engine: VectorE everywhere EXCEPT after the very last exp,
            # when ScalarE is idle (mid-stream, anything inserted into the
            # ScalarE FIFO delays the exp stream and stalls the PE via the
            # score-bank rotation -- measured +14us)
            cls = []
            for qs in range(4 * ich, 4 * ich + 4):
                for et in range(2):
                    def y_u(b=b, qs=qs, et=et, OT=OT):
                        acc = acc_ps.tile([128, 512], F32, name="yacc",
                                          tag="acc")
                        nc.tensor.matmul(
                            acc[:], OT[:, qs * 128:(qs + 1) * 128],
                            wo[:, et * 512:(et + 1) * 512],
                            start=True, stop=True)
                        yb = ybp.tile([128, 512], BF16, name="yb")
                        if final and (qs + et) % 2 == 0:
                            nc.scalar.activation(yb[:], acc[:], AF.Copy)
                        else:
                            nc.vector.tensor_copy(yb[:], acc[:])
                        nc.sync.dma_start(
                            ypd[b * S + qs * 128: b * S + (qs + 1) * 128,
                                et * 512:(et + 1) * 512], yb[:])
                    cls.append((C_OP, y_u))
            return cls

        # filler queue holds (cost_us, closure); pacing is cost-aware: each
        # j-iteration accrues a per-batch budget and pops closures until the
        # accrued deficit is spent, so PE work per j stays near-uniform
        # instead of lurching with the 1-or-2-closure slot scheme
        fillers = deque()
        C_QK, C_V, C_OP = 0.43, 0.27, 0.27

        def queue_mass():
            return sum(c for c, _ in fillers)

        def run_filler_budget(st, max_pops=None):
            # max_pops caps the closures popped this j while still accruing
            # budget (deficit carries): in the first js of each ich a BURST
            # of outproj fillers head-of-line blocks the ich's first score
            # pair on the cast-gated acc rotation (~1us, wait=1196), but a
            # single closure per j never stalls -- and fully deferring (0)
            # measured +8us worse from the j2-j4 bulge it creates
            st["deficit"] += st["per_j"]
            n = 0
            while (fillers and st["deficit"] > 0
                   and (max_pops is None or n < max_pops)):
                cost, fn = fillers.popleft()
                fn()
                st["deficit"] -= cost
                n += 1

        def drain_filler():
            while fillers:
                fillers.popleft()[1]()

        # ---- attention for batch b ----
        # Returns the batch's outproj closures for b < B-1 (deferred into the
        # NEXT batch's filler stream: batch B-1 has no proj filler of its own
        # and would otherwise starve the PE against the exp stream); for
        # b == B-1 outproj is appended per-ich with a one-ich lag as filler.
        def attention(b):
            qT, kT = qTs[b], kTs[b]
            vt4 = vt4_pair[b % 2]
            deferred = []
            last = b == B - 1
            planned = queue_mass() + (32 * C_OP if last else 0.0)
            # head start: the exp pipeline primes over the first couple of
            # j's, so the PE has extra filler cover at batch start; batch 3
            # (thin exp margin) gets more
            # (2.0, 0.8) measured best; NOTE bench runs during a device
            # throttle regime (ham type-1 windows 50+us) read ~480us for ANY
            # config -- don't tune pacing without checking ham first
            st = {"per_j": planned / (NICH * ST),
                  "deficit": 2.0 if last else 0.8}
            OT = otp.tile([128, S], BF16)
            OTs[b] = OT
            for ich in range(NICH):
                pvs = [pv_ps.tile([128, 512], F32, tag=f"pv{h}",
                                  name=f"pv{h}") for h in range(2)]
                prev = None
                for j in range(ST):
                    # PV(j-1) and filler EMIT BEFORE the j scores: a score
                    # pair stalling on the depth-2 sc-bank rotation would
                    # otherwise block this ready work behind it in the
                    # in-order PE FIFO -- this way the bank wait is absorbed
                    # by useful matmuls instead of idling the PE
                    if prev is not None:
                        pj, ppg = prev
                        for half in range(2):
                            nc.tensor.matmul(
                                pvs[half][:], vt4[:, pj, half, :],
                                ppg[:, half * 512:(half + 1) * 512],
                                start=(pj == 0), stop=False)
                    run_filler_budget(st, max_pops=1 if j < 3 else None)
                    sc = sc_ps.tile([128, 1024], F32)
                    for half in range(2):
                        p0 = half * 64
                        nc.tensor.matmul(
                            sc[:, half * 512:(half + 1) * 512],
                            kT[p0:p0 + 64, j * 128:(j + 1) * 128],
                            qT[p0:p0 + 64, ich * 512:(ich + 1) * 512],
                            start=True, stop=True)
                    pg = pgp.tile([128, 1024], BF16)
                    nc.scalar.activation(pg[:], sc[:], AF.Exp, scale=SCALE)
                    prev = (j, pg)
                pj, ppg = prev
                for half in range(2):
                    nc.tensor.matmul(
                        pvs[half][:], vt4[:, pj, half, :],
                        ppg[:, half * 512:(half + 1) * 512],
                        start=False, stop=True)
                # per-ich normalize, all reads straight from the PV PSUM
                # banks: fast ~18-bit reciprocal of the denominator row (64),
                # partition-broadcast it, scale rows 0:64 into OT.  The V bias
                # is folded into b_out on the host (sum_k P[k] = 1, so bv
                # contributes the constant W_out @ b_v to y).  On the very
                # last ich the denominator staging copies go to the idle
                # ScalarE (copy shares the exp ACT table) to shorten the
                # exposed vector-queue chain.
                final = last and ich == NICH - 1
                # per-half emission (copy+rcp+bcast adjacent) so half 0's
                # gpsimd broadcast launches one vector-op earlier and the PV
                # banks release sooner
                rbs = []
                for half in range(2):
                    dn1 = rcpp.tile([1, 512], F32, name="dn1")
                    if final:
                        nc.scalar.activation(dn1[:], pvs[half][64:65, :],
                                             AF.Copy)
                    else:
                        nc.vector.tensor_copy(dn1[:], pvs[half][64:65, :])
                    rcp1 = rcpp.tile([1, 512], F32, name="rcp1")
                    nc.vector.reciprocal_approx_fast(rcp1[:], dn1[:])
                    rb = rbp.tile([64, 512], F32)
                    nc.gpsimd.partition_broadcast(rb[:], rcp1[0:1, :])
                    rbs.append(rb)
                for half in range(2):
                    nc.vector.tensor_mul(
                        OT[half * 64:half * 64 + 64,
                           ich * 512:(ich + 1) * 512],
                        pvs[half][0:64, :], rbs[half])
                # out-projection for the PREVIOUS ich (lag keeps the PE from
                # reaching a y matmul before its OT chunk is normalized)
                if ich >= 1:
                    cls = make_outproj_closures(b, ich - 1)
                    (fillers.extend if last else deferred.extend)(cls)
            cls = make_outproj_closures(b, NICH - 1, final=last)
            (fillers.extend if last else deferred.extend)(cls)
            return deferred

        # ================= emission =================
        # prefetch x of batches 0 and 1 before batch 0's filler closures can
        # reach the PE FIFO, so proj(1) never stalls the attention(0) stream
        emit_x_dma(0, engines=[nc.sync, nc.scalar])
        emit_x_dma(1, engines=[nc.sync, nc.scalar])
        for dst, dsrc in ((wsv, wvd), (wo, wod)):
            nc.sync.dma_start(dst[:], dsrc)
        # dependency-free matmuls ramp the PE clock to full rate AND keep it
        # continuously busy until x(0) lands (idle drops the pstate again;
        # 24 was measured 4us worse, 64 delays proj(0))
        for i in range(40):
            wps = acc_ps.tile([128, 512], F32, name="wps", tag="acc")
            nc.tensor.matmul(wps[:], wrm[:, 0:128], wrm[:, 0:512],
                             start=True, stop=True)
        # pull the exp ACT table load into the startup DMA window so the
        # first real activation doesn't pay the ~1.3us table swap
        dummy_act = const.tile([128, 1], BF16)
        nc.scalar.activation(dummy_act[:], ident[:, 0:1], AF.Exp, scale=1.0)
        # fast start: only K, Q(rch0) and V(group 0) inline; the rest of
        # proj(0) becomes priority filler consumed by attention(0) in an
        # order matching when attention first needs each piece
        k0, q0, v0 = make_proj_closures(0)
        for _, cl in k0 + q0[0] + v0[0]:
            cl()
        fillers.extend(v0[1] + v0[2] + v0[3] + q0[1] + q0[2] + q0[3])
        deferred, held = [], []
        for b in range(B):
            if b + 1 < B:
                if b + 2 < B:
                    emit_x_dma(b + 2, engines=[nc.gpsimd])
                kc, qc, vc = make_proj_closures(b + 1)
                proj_cls = (kc + [c for r in qc for c in r]
                            + [c for g in vc for c in g])
            else:
                proj_cls = []
            # deferred outproj(b-1) runs as a contiguous block first (the
            # proj closures hold their PSUM acc across 4-8 closures, so
            # interleaving outproj between them serializes on the one free
            # acc buffer -- measured +60us), then proj(b+1).  Halves of
            # outproj(0) and outproj(1) are held back into batch 3's window:
            # batch 3 has no proj filler and runs exp-starved otherwise.
            take = 16 if b in (1, 2) else len(deferred)
            fillers.extend(deferred[:take])
            held.extend(deferred[take:])
            fillers.extend(proj_cls)
            if b == B - 1:
                fillers.extend(held)
            assert len(fillers) <= 132, (b, len(fillers))
            deferred = attention(b)
        drain_filler()

    nc.compile()
    return nc


_cache = {}


def _get_nc():
    if "nc" not in _cache:
        _cache["nc"] = build()
    return _cache["nc"]


def build_in_maps(x_q, W_qkv, b_qkv, W_out, b_out):
    x_q = np.ascontiguousarray(x_q, dtype=np.float32)
    W_qkv = np.asarray(W_qkv, dtype=np.float32)
    b_qkv = np.ascontiguousarray(b_qkv, dtype=np.float32)
    W_out = np.asarray(W_out, dtype=np.float32)
    bf = ml_dtypes.bfloat16
    # xTd[b, ct, p, s] = x[b, s, ct*128+p]
    xT = np.ascontiguousarray(
        x_q.transpose(0, 2, 1).reshape(B, CT, 128, S)).astype(bf)
    in_maps = []
    for c in range(NCORES):
        def wslice(off):
            # [p, ct*128+f] = W_qkv[off + c*128 + f, ct*128 + p]
            sl = W_qkv[off + c * 128: off + (c + 1) * 128, :]  # [f, chan]
            return np.ascontiguousarray(
                sl.reshape(128, CT, 128).transpose(2, 1, 0).reshape(
                    128, CT * 128)).astype(bf)
        wo = np.ascontiguousarray(W_out[:, c * 128:(c + 1) * 128].T).astype(bf)
        in_maps.append({
            "xT": xT,
            "wq": wslice(0),
            "wk": wslice(C),
            "wv": wslice(2 * C),
            "wo": wo,
            "bq": np.ascontiguousarray(
                b_qkv[c * 128:(c + 1) * 128].reshape(128, 1)),
            "bk": np.ascontiguousarray(
                b_qkv[C + c * 128: C + (c + 1) * 128].reshape(128, 1)),
        })
    return in_maps


def kernel(x_q, W_qkv, b_qkv, W_out, b_out):
    """Core c computes heads (2c, 2c+1) for all batches and the partial
    out-projection against W_out[:, c*128:(c+1)*128]; the host sums the
    8 partials and adds b_out (the tensor-parallel unshard)."""
    b_out = np.ascontiguousarray(b_out, dtype=np.float32)
    nc = _get_nc()
    in_maps = build_in_maps(x_q, W_qkv, b_qkv, W_out, b_out)
    res = run_bass_kernel_spmd(nc, in_maps, list(range(NCORES)))
    y = np.zeros((B * S, C), dtype=np.float32)
    for c in range(NCORES):
        y += np.asarray(res.results[c]["yp"]).astype(np.float32)
    # device OT is the normalized attention WITHOUT the V bias; since the
    # softmax rows sum to 1, bv contributes the token-independent constant
    # W_out @ b_v to y -- fold it into the output bias here
    bv_full = np.ascontiguousarray(b_qkv, dtype=np.float32)[2 * C:3 * C]
    y += (b_out + np.asarray(W_out, dtype=np.float32) @ bv_full)[None, :]
    return y.reshape(B, S, C)


if __name__ == "__main__":
    rng = np.random.default_rng(0)
    x_q = rng.standard_normal((B, S, C), dtype=np.float32)
    s = 1.0 / np.sqrt(C)
    W_qkv = rng.uniform(-s, s, (3 * C, C)).astype(np.float32)
    b_qkv = rng.uniform(-s, s, 3 * C).astype(np.float32)
    W_out = rng.uniform(-s, s, (C, C)).astype(np.float32)
    b_out = rng.uniform(-s, s, C).astype(np.float32)
    got = kernel(x_q=x_q, W_qkv=W_qkv, b_qkv=b_qkv, W_out=W_out, b_out=b_out)
    print("smoke ok", got.shape, float(np.abs(got).max()))



# revision 104
# speedup vs baseline: 1.0060x; 1.0060x over previous
"""Multi-head attention (B=4, S=2048, C=1024, H=16) on 8 TRN2 NeuronCores.

Tensor-parallel over heads: core c owns head pair (2c, 2c+1) for ALL 4
batches and computes a partial out-projection y_c = O_c @ W_out[:, c*128:
(c+1)*128].T; the host sums the 8 partials and adds b_out + W_out @ b_v
(the V bias commutes through softmax+outproj since rows of P sum to 1).

Per core, everything is bf16 (PSUM accumulation in fp32) and SBUF-resident
per batch; x streams in per batch, double-buffered, 16 descriptors per
batch so one batch saturates all DMA queues before the next queues behind.
The per-batch pipeline keeps ScalarE (exp) saturated while the PE
interleaves, as cost-budgeted filler between score/PV blocks, the NEXT
batch's QKV projection and earlier batches' out-projections (deferred one
batch, with halves of outproj(0..1) held into batch 3's window, which has
no proj filler of its own):

  proj(b): qT/kT feature-major [128 pair-feats, 2048 rows] via W-stationary
  matmuls; V keys-major via x-stationary matmuls into a padded stationary
  layout [V_h(64) | 1 | 0*63] per (key-tile, head) so the PV matmul also
  accumulates the softmax denominator at out row 64.

  attention(b, ich of 512 q): per key-tile j: row-packed pair scores
  (head A on PE rows 0-63, head B on 64-127; the hardware runs them as
  concurrent 64-row groups, ~385ns/pair) into a double-buffered 2-bank
  PSUM tile -> one exp (scale=0.125 folded in) -> PV of tile j-1 pipelined
  behind the exp.  Per-ich normalize straight from the PV PSUM banks:
  VectorE copy of the denominator row, reciprocal_approx_fast ([1,512],
  ~18 bits), gpsimd partition_broadcast, one tensor mul into OT.  On the
  very last ich the denominator copies ride the then-idle ScalarE and the
  outproj casts alternate ScalarE/VectorE to shorten the exposed tail.

PSUM (8 banks): scores 2x2, PV accumulators 2x1, proj/outproj acc 2x1.
Mid-stream engine discipline (measured, not guessed): ScalarE's FIFO is
reserved for the exp stream (inserting copies/bias-adds costs ~14us in
score-bank stalls); deferred outproj runs as contiguous blocks because
proj closures hold PSUM accs across 4-8 closures (+60us if interleaved).
"""

from collections import deque
from contextlib import ExitStack

import numpy as np
import ml_dtypes

import concourse.mybir as mybir
import concourse.tile as tile
from concourse import bacc
from concourse.bass_utils import run_bass_kernel_spmd
from concourse.masks import make_identity

F32 = mybir.dt.float32
BF16 = mybir.dt.bfloat16
AF = mybir.ActivationFunctionType

B, S, C, H, DH = 4, 2048, 1024, 16, 64
NCORES = 8
SCALE = DH ** -0.5  # 0.125
CT = C // 128  # 8 channel tiles
ST = S // 128  # 16 key tiles
NICH = S // 512  # 4 query chunks per batch


def build():
    nc = bacc.Bacc("TRN2", target_bir_lowering=False, debug=False,
                   num_devices=NCORES)

    # host-prepared layouts (pure data movement on the host):
    #   xTd[b, ct, p, s] = x[b, s, ct*128+p]
    #   wq[p, ct*128+f] = W_qkv[c*128+f, ct*128+p]          (this core's Q)
    #   wk / wv same with row offsets 1024+c*128 / 2048+c*128
    #   wo[p, ch] = W_out[ch, c*128+p]
    #   bq/bk/bv[p, 0] = b_qkv[(0|1024|2048) + c*128 + p]
    xTd = nc.dram_tensor("xT", [B, CT, 128, S], BF16, kind="ExternalInput").ap()
    wqd = nc.dram_tensor("wq", [128, CT * 128], BF16, kind="ExternalInput").ap()
    wkd = nc.dram_tensor("wk", [128, CT * 128], BF16, kind="ExternalInput").ap()
    wvd = nc.dram_tensor("wv", [128, CT * 128], BF16, kind="ExternalInput").ap()
    wod = nc.dram_tensor("wo", [128, C], BF16, kind="ExternalInput").ap()
    bqd = nc.dram_tensor("bq", [128, 1], F32, kind="ExternalInput").ap()
    bkd = nc.dram_tensor("bk", [128, 1], F32, kind="ExternalInput").ap()
    ypd = nc.dram_tensor("yp", [B * S, C], BF16, kind="ExternalOutput").ap()

    with tile.TileContext(nc) as tc, ExitStack() as ctx:
        const = ctx.enter_context(tc.tile_pool(name="const", bufs=1))
        ident = const.tile([128, 128], F32)
        make_identity(nc, ident[:])
        wsq = const.tile([128, CT * 128], BF16)
        wsk = const.tile([128, CT * 128], BF16)
        wsv = const.tile([128, CT * 128], BF16)
        wo = const.tile([128, C], BF16)
        bq = const.tile([128, 1], F32)
        bk = const.tile([128, 1], F32)
        for dst, src in ((wsk, wkd), (wsq, wqd), (bq, bqd), (bk, bkd)):
            nc.sync.dma_start(dst[:], src)

        # warmup tile for PE clock ramp during the startup DMA window
        wrm = const.tile([128, 512], BF16)
        for i in range(4):
            nc.vector.tensor_scalar(
                wrm[:, i * 128:(i + 1) * 128], ident[:, 0:128],
                0.0, 0.0, mybir.AluOpType.mult, mybir.AluOpType.mult)

        # persistent V tiles (ping-pong): ones/zeros pad written once
        vt_pair = [const.tile([128, ST * 256], BF16, name=f"vt{i}")
                   for i in range(2)]
        vt4_pair = []
        for vt in vt_pair:
            vt4 = vt[:].rearrange("p (t g f) -> p t g f", g=2, f=128)
            vt4_pair.append(vt4)
            nc.vector.tensor_scalar(
                vt4[:, :, :, DH:DH + 1],
                ident[:, 0:2 * ST].rearrange("p (t g) -> p t g", g=2),
                0.0, 1.0, mybir.AluOpType.mult, mybir.AluOpType.add)
            for g in range(2):
                nc.vector.tensor_scalar(
                    vt4[:, :, g:g + 1, DH + 1:128],
                    wsk[:, 0:ST * (127 - DH)].rearrange(
                        "p (t g f) -> p t g f", g=1, f=127 - DH),
                    0.0, 0.0, mybir.AluOpType.mult, mybir.AluOpType.mult)

        xp = ctx.enter_context(tc.tile_pool(name="xp", bufs=2))
        qp = ctx.enter_context(tc.tile_pool(name="qp", bufs=2))
        kp = ctx.enter_context(tc.tile_pool(name="kp", bufs=2))
        # bufs=4: all four OT tiles stay alive (outproj(0),(1) halves are
        # held back into batch 3's filler window)
        otp = ctx.enter_context(tc.tile_pool(name="otp", bufs=4))
        pgp = ctx.enter_context(tc.tile_pool(name="pgp", bufs=5))
        rcpp = ctx.enter_context(tc.tile_pool(name="rcpp", bufs=8))
        rbp = ctx.enter_context(tc.tile_pool(name="rbp", bufs=6))
        # deep yb pool: y DMAs share the 16 round-robin queues with the 4MB
        # x prefetches, so several y writes can be in flight behind a
        # prefetch segment; a shallow pool stalls the cast->acc rotation
        ybp = ctx.enter_context(tc.tile_pool(name="ybp", bufs=12))
        sc_ps = ctx.enter_context(
            tc.tile_pool(name="sc_ps", bufs=2, space="PSUM"))
        pv_ps = ctx.enter_context(
            tc.tile_pool(name="pv_ps", bufs=1, space="PSUM"))
        acc_ps = ctx.enter_context(
            tc.tile_pool(name="acc_ps", bufs=2, space="PSUM"))

        xts = [None] * (B + 1)
        qTs = [None] * B
        kTs = [None] * B
        OTs = [None] * B

        def emit_x_dma(b, engines=None):
            # 16 descriptors spread one batch's x across all 16 round-robin
            # DMA queues.  Descriptor ENQUEUE itself costs ~0.6us on the
            # issuing engine, so the startup batches alternate between the
            # two hardware-DGE engines (Sync + the then-idle ScalarE) and
            # mid-stream prefetches go through gpsimd's software DGE to keep
            # Sync free for the latency-sensitive y-output descriptors.
            engines = engines or [nc.sync]
            xt = xp.tile([128, CT * S], BF16)
            xts[b] = xt
            n = 0
            for ct in range(CT):
                for hf in range(2):
                    lo = ct * S + hf * (S // 2)
                    engines[n % len(engines)].dma_start(
                        xt[:, lo:lo + S // 2],
                        xTd[b, ct, :, hf * (S // 2):(hf + 1) * (S // 2)])
                    n += 1

        # ---- projection of batch b, as lists of PE-sized closures ----
        def make_proj_closures(b):
            xt = xts[b]
            qT = qp.tile([128, S], BF16)
            kT = kp.tile([128, S], BF16)
            qTs[b], kTs[b] = qT, kT
            vt4 = vt4_pair[b % 2]

            # Q/K: per (which, rch) one 8-ct accumulation, split into 4
            # ~0.45us closures so the filler pacing stays fine-grained
            def qk_quads(wt, dst, bias, rch):
                st8 = {}
                def qk_u(q4):
                    def f():
                        if q4 == 0:
                            st8["acc"] = acc_ps.tile([128, 512], F32,
                                                     name="qkacc", tag="acc")
                        acc = st8["acc"]
                        for ct in range(2 * q4, 2 * q4 + 2):
                            nc.tensor.matmul(
                                acc[:], wt[:, ct * 128:(ct + 1) * 128],
                                xt[:, ct * S + rch * 512: ct * S + rch * 512 + 512],
                                start=(ct == 0), stop=(ct == 7))
                        if q4 == 3:
                            nc.vector.tensor_scalar_add(
                                dst[:, rch * 512:(rch + 1) * 512], acc[:],
                                bias[:, 0:1])
                    return f
                return [qk_u(q4) for q4 in range(4)]

            k_cls = [(C_QK, c)
                     for rch in range(4) for c in qk_quads(wsk, kT, bk, rch)]
            q_cls = [[(C_QK, c) for c in qk_quads(wsq, qT, bq, rch)]
                     for rch in range(4)]

            # V: per group of 4 key-tiles, 8 closures of 4-ct halves
            def v_group(g4):
                st8 = {}
                def v_u(u, h):
                    def f():
                        if u == 0 and h == 0:
                            st8["acc"] = acc_ps.tile([128, 512], F32,
                                                     name="vacc", tag="acc")
                        acc = st8["acc"]
                        kt = g4 * 4 + u
                        for ct in range(4 * h, 4 * h + 4):
                            nc.tensor.matmul(
                                acc[:, u * 128:(u + 1) * 128],
                                xt[:, ct * S + kt * 128: ct * S + (kt + 1) * 128],
                                wsv[:, ct * 128:(ct + 1) * 128],
                                start=(ct == 0), stop=(ct == CT - 1))
                        if u == 3 and h == 1:
                            nc.vector.tensor_copy(
                                vt4[:, g4 * 4:(g4 + 1) * 4, :, 0:DH],
                                acc[:].rearrange("p (u g f) -> p u g f",
                                                 u=4, f=DH))
                    return f
                return [v_u(u, h) for u in range(4) for h in range(2)]

            v_cls = [[(C_V, c) for c in v_group(g4)] for g4 in range(4)]
            return k_cls, q_cls, v_cls

        # ---- out-projection of one ich of batch b as closures ----
        # final=True (very last ich): alternate the PSUM->SBUF casts between
        # ScalarE (idle after the last exp; Copy shares the exp table) and
        # VectorE so the 2-deep acc rotation isn't serialized on one engine.
        def make_outproj_closures(b, ich, final=False):
            OT = OTs[b]
            # Cast engine: VectorE everywhere EXCEPT after the very last exp,
            # when ScalarE is idle (mid-stream, anything inserted into the
            # ScalarE FIFO delays the exp stream and stalls the PE via the
            # score-bank rotation -- measured +14us)
            cls = []
            for qs in range(4 * ich, 4 * ich + 4):
                for et in range(2):
                    def y_u(b=b, qs=qs, et=et, OT=OT):
                        acc = acc_ps.tile([128, 512], F32, name="yacc",
                                          tag="acc")
                        nc.tensor.matmul(
                            acc[:], OT[:, qs * 128:(qs + 1) * 128],
                            wo[:, et * 512:(et + 1) * 512],
                            start=True, stop=True)
                        yb = ybp.tile([128, 512], BF16, name="yb")
                        if final and (qs + et) % 2 == 0:
                            nc.scalar.activation(yb[:], acc[:], AF.Copy)
                        else:
                            nc.vector.tensor_copy(yb[:], acc[:])
                        nc.sync.dma_start(
                            ypd[b * S + qs * 128: b * S + (qs + 1) * 128,
                                et * 512:(et + 1) * 512], yb[:])
                    cls.append((C_OP, y_u))
            return cls

        # filler queue holds (cost_us, closure); pacing is cost-aware: each
        # j-iteration accrues a per-batch budget and pops closures until the
        # accrued deficit is spent, so PE work per j stays near-uniform
        # instead of lurching with the 1-or-2-closure slot scheme
        fillers = deque()
        C_QK, C_V, C_OP = 0.43, 0.27, 0.27

        def queue_mass():
            return sum(c for c, _ in fillers)

        def run_filler_budget(st):
            st["deficit"] += st["per_j"]
            while fillers and st["deficit"] > 0:
                cost, fn = fillers.popleft()
                fn()
                st["deficit"] -= cost

        def drain_filler():
            while fillers:
                fillers.popleft()[1]()

        # ---- attention for batch b ----
        # Returns the batch's outproj closures for b < B-1 (deferred into the
        # NEXT batch's filler stream: batch B-1 has no proj filler of its own
        # and would otherwise starve the PE against the exp stream); for
        # b == B-1 outproj is appended per-ich with a one-ich lag as filler.
        def attention(b):
            qT, kT = qTs[b], kTs[b]
            vt4 = vt4_pair[b % 2]
            deferred = []
            last = b == B - 1
            planned = queue_mass() + (32 * C_OP if last else 0.0)
            # head start: the exp pipeline primes over the first couple of
            # j's, so the PE has extra filler cover at batch start; batch 3
            # (thin exp margin) gets more
            # (2.0, 0.8) measured best; NOTE bench runs during a device
            # throttle regime (ham type-1 windows 50+us) read ~480us for ANY
            # config -- don't tune pacing without checking ham first
            st = {"per_j": planned / (NICH * ST),
                  "deficit": 2.0 if last else 0.8}
            OT = otp.tile([128, S], BF16)
            OTs[b] = OT
            for ich in range(NICH):
                pvs = [pv_ps.tile([128, 512], F32, tag=f"pv{h}",
                                  name=f"pv{h}") for h in range(2)]
                prev = None
                for j in range(ST):
                    # PV(j-1) and filler EMIT BEFORE the j scores: a score
                    # pair stalling on the depth-2 sc-bank rotation would
                    # otherwise block this ready work behind it in the
                    # in-order PE FIFO -- this way the bank wait is absorbed
                    # by useful matmuls instead of idling the PE
                    if prev is not None:
                        pj, ppg = prev
                        for half in range(2):
                            nc.tensor.matmul(
                                pvs[half][:], vt4[:, pj, half, :],
                                ppg[:, half * 512:(half + 1) * 512],
                                start=(pj == 0), stop=False)
                    run_filler_budget(st)
                    sc = sc_ps.tile([128, 1024], F32)
                    for half in range(2):
                        p0 = half * 64
                        nc.tensor.matmul(
                            sc[:, half * 512:(half + 1) * 512],
                            kT[p0:p0 + 64, j * 128:(j + 1) * 128],
                            qT[p0:p0 + 64, ich * 512:(ich + 1) * 512],
                            start=True, stop=True)
                    pg = pgp.tile([128, 1024], BF16)
                    nc.scalar.activation(pg[:], sc[:], AF.Exp, scale=SCALE)
                    prev = (j, pg)
                pj, ppg = prev
                for half in range(2):
                    nc.tensor.matmul(
                        pvs[half][:], vt4[:, pj, half, :],
                        ppg[:, half * 512:(half + 1) * 512],
                        start=False, stop=True)
                # per-ich normalize, all reads straight from the PV PSUM
                # banks: fast ~18-bit reciprocal of the denominator row (64),
                # partition-broadcast it, scale rows 0:64 into OT.  The V bias
                # is folded into b_out on the host (sum_k P[k] = 1, so bv
                # contributes the constant W_out @ b_v to y).  On the very
                # last ich the denominator staging copies go to the idle
                # ScalarE (copy shares the exp ACT table) to shorten the
                # exposed vector-queue chain.
                final = last and ich == NICH - 1
                # per-half emission (copy+rcp+bcast adjacent) so half 0's
                # gpsimd broadcast launches one vector-op earlier and the PV
                # banks release sooner
                rbs = []
                for half in range(2):
                    dn1 = rcpp.tile([1, 512], F32, name="dn1")
                    if final:
                        nc.scalar.activation(dn1[:], pvs[half][64:65, :],
                                             AF.Copy)
                    else:
                        nc.vector.tensor_copy(dn1[:], pvs[half][64:65, :])
                    rcp1 = rcpp.tile([1, 512], F32, name="rcp1")
                    nc.vector.reciprocal_approx_fast(rcp1[:], dn1[:])
                    rb = rbp.tile([64, 512], F32)
                    nc.gpsimd.partition_broadcast(rb[:], rcp1[0:1, :])
                    rbs.append(rb)
                for half in range(2):
                    nc.vector.tensor_mul(
                        OT[half * 64:half * 64 + 64,
                           ich * 512:(ich + 1) * 512],
                        pvs[half][0:64, :], rbs[half])
                # out-projection for the PREVIOUS ich (lag keeps the PE from
                # reaching a y matmul before its OT chunk is normalized)
                if ich >= 1:
                    cls = make_outproj_closures(b, ich - 1)
                    (fillers.extend if last else deferred.extend)(cls)
            cls = make_outproj_closures(b, NICH - 1, final=last)
            (fillers.extend if last else deferred.extend)(cls)
            return deferred

        # ================= emission =================
        # prefetch x of batches 0 and 1 before batch 0's filler closures can
        # reach the PE FIFO, so proj(1) never stalls the attention(0) stream
        emit_x_dma(0, engines=[nc.sync, nc.scalar])
        emit_x_dma(1, engines=[nc.sync, nc.scalar])
        for dst, dsrc in ((wsv, wvd), (wo, wod)):
            nc.sync.dma_start(dst[:], dsrc)
        # dependency-free matmuls ramp the PE clock to full rate AND keep it
        # continuously busy until x(0) lands (idle drops the pstate again;
        # 24 was measured 4us worse, 64 delays proj(0))
        for i in range(40):
            wps = acc_ps.tile([128, 512], F32, name="wps", tag="acc")
            nc.tensor.matmul(wps[:], wrm[:, 0:128], wrm[:, 0:512],
                             start=True, stop=True)
        # pull the exp ACT table load into the startup DMA window so the
        # first real activation doesn't pay the ~1.3us table swap
        dummy_act = const.tile([128, 1], BF16)
        nc.scalar.activation(dummy_act[:], ident[:, 0:1], AF.Exp, scale=1.0)
        # fast start: only K, Q(rch0) and V(group 0) inline; the rest of
        # proj(0) becomes priority filler consumed by attention(0) in an
        # order matching when attention first needs each piece
        k0, q0, v0 = make_proj_closures(0)
        for _, cl in k0 + q0[0] + v0[0]:
            cl()
        fillers.extend(v0[1] + v0[2] + v0[3] + q0[1] + q0[2] + q0[3])
        deferred, held = [], []
        for b in range(B):
            if b + 1 < B:
                if b + 2 < B:
                    emit_x_dma(b + 2, engines=[nc.gpsimd])
                kc, qc, vc = make_proj_closures(b + 1)
                proj_cls = (kc + [c for r in qc for c in r]
                            + [c for g in vc for c in g])
            else:
                proj_cls = []
            # deferred outproj(b-1) runs as a contiguous block first (the
            # proj closures hold their PSUM acc across 4-8 closures, so
            # interleaving outproj between them serializes on the one free
            # acc buffer -- measured +60us), then proj(b+1).  Halves of
            # outproj(0) and outproj(1) are held back into batch 3's window:
            # batch 3 has no proj filler and runs exp-starved otherwise.
            take = 16 if b in (1, 2) else len(deferred)
            fillers.extend(deferred[:take])
            held.extend(deferred[take:])
            fillers.extend(proj_cls)
            if b == B - 1:
                fillers.extend(held)
            assert len(fillers) <= 132, (b, len(fillers))
            deferred = attention(b)
        drain_filler()

    nc.compile()
    return nc


_cache = {}


def _get_nc():
    if "nc" not in _cache:
        _cache["nc"] = build()
    return _cache["nc"]


def build_in_maps(x_q, W_qkv, b_qkv, W_out, b_out):
    x_q = np.ascontiguousarray(x_q, dtype=np.float32)
    W_qkv = np.asarray(W_qkv, dtype=np.float32)
    b_qkv = np.ascontiguousarray(b_qkv, dtype=np.float32)
    W_out = np.asarray(W_out, dtype=np.float32)
    bf = ml_dtypes.bfloat16
    # xTd[b, ct, p, s] = x[b, s, ct*128+p]
    xT = np.ascontiguousarray(
        x_q.transpose(0, 2, 1).reshape(B, CT, 128, S)).astype(bf)
    in_maps = []
    for c in range(NCORES):
        def wslice(off):
            # [p, ct*128+f] = W_qkv[off + c*128 + f, ct*128 + p]
            sl = W_qkv[off + c * 128: off + (c + 1) * 128, :]  # [f, chan]
            return np.ascontiguousarray(
                sl.reshape(128, CT, 128).transpose(2, 1, 0).reshape(
                    128, CT * 128)).astype(bf)
        wo = np.ascontiguousarray(W_out[:, c * 128:(c + 1) * 128].T).astype(bf)
        in_maps.append({
            "xT": xT,
            "wq": wslice(0),
            "wk": wslice(C),
            "wv": wslice(2 * C),
            "wo": wo,
            "bq": np.ascontiguousarray(
                b_qkv[c * 128:(c + 1) * 128].reshape(128, 1)),
            "bk": np.ascontiguousarray(
                b_qkv[C + c * 128: C + (c + 1) * 128].reshape(128, 1)),
        })
    return in_maps


def kernel(x_q, W_qkv, b_qkv, W_out, b_out):
    """Core c computes heads (2c, 2c+1) for all batches and the partial
    out-projection against W_out[:, c*128:(c+1)*128]; the host sums the
    8 partials and adds b_out (the tensor-parallel unshard)."""
    b_out = np.ascontiguousarray(b_out, dtype=np.float32)
    nc = _get_nc()
    in_maps = build_in_maps(x_q, W_qkv, b_qkv, W_out, b_out)
    res = run_bass_kernel_spmd(nc, in_maps, list(range(NCORES)))
    y = np.zeros((B * S, C), dtype=np.float32)
    for c in range(NCORES):
        y += np.asarray(res.results[c]["yp"]).astype(np.float32)
    # device OT is the normalized attention WITHOUT the V bias; since the
    # softmax rows sum to 1, bv contributes the token-independent constant
    # W_out @ b_v to y -- fold it into the output bias here
    bv_full = np.ascontiguousarray(b_qkv, dtype=np.float32)[2 * C:3 * C]
    y += (b_out + np.asarray(W_out, dtype=np.float32) @ bv_full)[None, :]
    return y.reshape(B, S, C)


if __name__ == "__main__":
    rng = np.random.default_rng(0)
    x_q = rng.standard_normal((B, S, C), dtype=np.float32)
    s = 1.0 / np.sqrt(C)
    W_qkv = rng.uniform(-s, s, (3 * C, C)).astype(np.float32)
    b_qkv = rng.uniform(-s, s, 3 * C).astype(np.float32)
    W_out = rng.uniform(-s, s, (C, C)).astype(np.float32)
    b_out = rng.uniform(-s, s, C).astype(np.float32)
    got = kernel(x_q=x_q, W_qkv=W_qkv, b_qkv=b_qkv, W_out=W_out, b_out=b_out)
    print("smoke ok", got.shape, float(np.abs(got).max()))

